# revision 25
# baseline (speedup 1.0000x reference)
"""Trainium2 Bass kernel for a dense transformer block (B=8, T=1024, C=1024, H=16).

Data-parallel over batch across the 8 NeuronCores (one batch element per core,
weights replicated, no collectives).

Per-core dataflow (activations feature-major ("transposed") for matmuls,
token-major ("natural") for layernorm / softmax-denominator work):

  x [T,C] f32 --LN1(stats on DVE, normalize on ACT)--> h bf16 --PE T--> h1T
  v    = h @ Wv          (lhsT=h1T chunks, rhs=Wv)    -> natural [T, C] bf16
  qkT  = (h @ Wqk)^T     (lhsT=Wqk, rhs=h1T)          -> [2C, T] bf16
         chunks for head-pairs 0-3 upfront; chunks 4-7 interleaved into the
         attention loop as tensor-engine filler under the ACT-bound exp work.
  S^T  = k q^T           (lhsT=kT, rhs=qT, K=D=64)    -> [tk, tq] psum f32
         both heads of a pair issued back-to-back on 64-row PE tiles
         (tile_position (0,0)/(64,0)) so they stream concurrently.
  E^T  = exp(S^T/8) bf16 (no max-sub; scores ~N(0,1)); causal diag masked
         post-exp with a 0/1 upper-tri mask.
  y    = P @ [v | 1]     (lhsT=E^T blk, rhs=v_aug)    -> [65, tq] psum,
         lagged one pair behind scores so PE stays busy while ACT runs exp;
         col 64 = softmax denominator; divide via DRAM-broadcast recip.
  r1   = x + y @ Wp      (lhsT=yT)                    -> natural f32
  h2   = LN2(r1) --transpose--> h2T bf16
  aT   = gelu_tanh(Wfc^T h2)   (lhsT=Wfc, rhs=h2T)    -> [4C, T] bf16
  out  = r1 + a @ Wmlp   (lhsT=aT, rhs=Wmlp halves)   -> natural [T, C] f32
         two column passes, 8x 1-bank PSUMs, half the Wmlp DMA traffic.

All matmuls bf16 (full PE rate) with fp32 PSUM accumulation; LN statistics and
residual stream stay fp32.
"""
import sys

sys.path.insert(0, "/opt/trn_rl_repo")

import numpy as np
import ml_dtypes

import concourse.bass as bass
import concourse.tile as tile
from concourse import mybir
from concourse.masks import make_identity
from concourse.vector_clock import ScopedClock

F32 = mybir.dt.float32
BF16 = mybir.dt.bfloat16
AF = mybir.ActivationFunctionType

T, C, H, D = 1024, 1024, 16, 64
NT = T // 128   # 8 token chunks
NC_ = C // 128  # 8 feature chunks
EPS = 1e-5

# ---------------------------------------------------------------------------
# Walrus in this container rejects >1 sem-wait per CTRL instruction; split the
# Tile tail-drain's waits across nop carriers.
_MAX_WAITS = 1


def _patched_drain_and_barrier(self, tick_clock, wait_clock):
    nc = self.nc
    carrier = nc.sync.nop(nofuse=True)
    wait_clock.add_sem_waits(carrier.ins, ScopedClock({None: tick_clock.global_clock}))
    si = carrier.ins.sync_info
    waits = list(si.on_wait) if si and si.on_wait else []
    if len(waits) > _MAX_WAITS:
        si.on_wait = waits[:_MAX_WAITS]
        for k in range(_MAX_WAITS, len(waits), _MAX_WAITS):
            extra = nc.sync.nop(nofuse=True)
            esi = extra.ins.sync_info
            if esi is None:
                extra.ins.sync_info = mybir.SyncInfo(
                    on_wait=waits[k:k + _MAX_WAITS], on_update=[]
                )
            else:
                esi.on_wait = waits[k:k + _MAX_WAITS]
    nc.sync.drain()
    nc.all_engine_barrier()
    popped = nc._tile_sem_poison_stack.pop()
    assert popped is self._sem_poison
    nc.clear_and_free_semaphores(list(self.sems.allocated().values()))
    nc.all_engine_barrier()


tile.TileContext._drain_and_barrier = _patched_drain_and_barrier


def _split_sync_waits(nc, max_waits=1):
    """Walrus here rejects >1 sem-wait per instruction; hoist extras onto
    preceding same-engine nops."""
    ctr = 0
    for f in nc.m.functions:
        for b in f.blocks:
            out = []
            for ins in b.instructions:
                si = ins.sync_info
                ws = list(si.on_wait) if si and si.on_wait else []
                if len(ws) > max_waits:
                    extra, keep = ws[:-max_waits], ws[-max_waits:]
                    for i in range(0, len(extra), max_waits):
                        nop = mybir.InstNoOp(
                            name=f"wsplit-{ctr}", engine=ins.engine,
                            sync_info=mybir.SyncInfo(
                                on_wait=extra[i:i + max_waits], on_update=[]))
                        ctr += 1
                        out.append(nop)
                    si.on_wait = keep
                out.append(ins)
            b.instructions = out


def build_nc(flags):
    nc = bass.Bass()

    x_d = nc.dram_tensor("x", [T, C], F32, kind="ExternalInput")
    # host-prearranged: [m_chunk, p, ko, 128] so per-chunk DMAs are contiguous
    wqk_d = nc.dram_tensor("w_qk", [2 * NC_, 128, NC_, 128], BF16,
                           kind="ExternalInput")
    wfc_d = nc.dram_tensor("w_fc", [4 * NC_, 128, NC_, 128], BF16,
                           kind="ExternalInput")
    wv_d = nc.dram_tensor("w_v", [C, C], BF16, kind="ExternalInput")
    wp_d = nc.dram_tensor("w_proj", [C, C], BF16, kind="ExternalInput")
    # host-prearranged: [half, k, p, 512] column halves for the 2-pass fc2
    wmlp_d = nc.dram_tensor("w_mlp", [2, 4 * NC_, 128, 512], BF16,
                            kind="ExternalInput")
    mask_d = nc.dram_tensor("mask_ut", [128, 128], BF16, kind="ExternalInput")
    opt = {}
    if flags["b_qk"]:
        opt["b_qk"] = nc.dram_tensor("b_qk", [128, 2 * NC_], F32, kind="ExternalInput")
    if flags["b_v"]:
        opt["b_v"] = nc.dram_tensor("b_v", [C], F32, kind="ExternalInput")
    if flags["b_proj"]:
        opt["b_proj"] = nc.dram_tensor("b_proj", [C], F32, kind="ExternalInput")
    if flags["b_fc"]:
        opt["b_fc"] = nc.dram_tensor("b_fc", [128, 4 * NC_], F32, kind="ExternalInput")
    if flags["b_mlp"]:
        opt["b_mlp"] = nc.dram_tensor("b_mlp", [C], F32, kind="ExternalInput")
    for nm in ("ln1_g", "ln1_b", "ln2_g", "ln2_b"):
        if flags[nm]:
            opt[nm] = nc.dram_tensor(nm, [C], F32, kind="ExternalInput")
    out_d = nc.dram_tensor("out", [T, C], F32, kind="ExternalOutput")

    with tile.TileContext(nc) as tc:
        _build_body(nc, tc, flags, x_d, wqk_d, wv_d, wp_d, wfc_d, wmlp_d,
                    mask_d, opt, out_d)
    _split_sync_waits(nc)
    return nc


def _build_body(nc, tc, flags, x_d, wqk_d, wv_d, wp_d, wfc_d, wmlp_d,
                mask_d, opt, out_d):
    from contextlib import ExitStack

    ctx = ExitStack()
    with ctx:
        const = ctx.enter_context(tc.tile_pool(name="const", bufs=1))
        big = ctx.enter_context(tc.tile_pool(name="big", bufs=1))
        scratch = ctx.enter_context(tc.tile_pool(name="scratch", bufs=2))
        small = ctx.enter_context(tc.tile_pool(name="small", bufs=8))
        o_pool = ctx.enter_context(tc.tile_pool(name="opool", bufs=2))
        dram = ctx.enter_context(tc.tile_pool(name="dram", bufs=1, space="DRAM"))

        # ---- constants -----------------------------------------------------
        ident = const.tile([128, 128], BF16, tag="ident")
        make_identity(nc, ident)
        mask_sb = const.tile([128, 128], BF16, tag="mask")
        nc.sync.dma_start(mask_sb[:], mask_d[:])
        eps_t = const.tile([128, 1], F32, tag="eps")
        nc.vector.memset(eps_t[:], EPS)

        def rep128(vec_dram):
            t = const.tile([128, C], F32, tag=f"rep_{vec_dram.tensor.name}")
            src = bass.AP(tensor=vec_dram.tensor, offset=0, ap=[[0, 128], [1, C]])
            nc.gpsimd.dma_start(out=t[:], in_=src)
            return t

        ln1_g_rep = rep128(opt["ln1_g"]) if flags["ln1_g"] else None
        ln1_b_rep = rep128(opt["ln1_b"]) if flags["ln1_b"] else None
        ln2_g_rep = rep128(opt["ln2_g"]) if flags["ln2_g"] else None
        ln2_b_rep = rep128(opt["ln2_b"]) if flags["ln2_b"] else None
        bv_rep = rep128(opt["b_v"]) if flags["b_v"] else None
        bproj_rep = rep128(opt["b_proj"]) if flags["b_proj"] else None
        bmlp_rep = rep128(opt["b_mlp"]) if flags["b_mlp"] else None
        bqk_sb = None
        if flags["b_qk"]:
            bqk_sb = const.tile([128, 2 * NC_], F32, tag="bqk")
            nc.sync.dma_start(bqk_sb[:], opt["b_qk"][:])
        bfc_sb = None
        if flags["b_fc"]:
            bfc_sb = const.tile([128, 4 * NC_], F32, tag="bfc")
            nc.sync.dma_start(bfc_sb[:], opt["b_fc"][:])

        # ---- persistent tiles ---------------------------------------------
        x_sb = big.tile([128, NT, C], F32, tag="x")        # x, then r1 in place
        bufT = big.tile([128, NC_, T], BF16, tag="bufT")   # h1T -> h2T
        yT = big.tile([128, NC_, T], BF16, tag="yT")       # attention out^T
        # all qkT chunks: [p, sub(q=0,k=1), pair, t]
        qk_full = big.tile([128, 2, NC_, T], BF16, tag="qkf")

        def layernorm_chunk(src_slice, g_rep, b_rep):
            stats = small.tile([128, 2, 6], F32, tag="bn_stats")
            xr = src_slice.rearrange("p (s f) -> p s f", f=512)
            for s in range(2):
                nc.vector.bn_stats(out=stats[:, s, :], in_=xr[:, s, :])
            mv = small.tile([128, 2], F32, tag="bn_mv")
            nc.vector.bn_aggr(out=mv[:], in_=stats[:])
            rstd = small.tile([128, 1], F32, tag="rstd")
            nc.scalar.activation(out=rstd[:], in_=mv[:, 1:2], func=AF.Sqrt,
                                 bias=eps_t[:], scale=1.0)
            nc.vector.reciprocal(out=rstd[:], in_=rstd[:])
            # nmr = -mu * rstd; normalize on ACT: h = x*rstd + nmr
            nmr = small.tile([128, 1], F32, tag="nmr")
            nc.vector.tensor_scalar(
                out=nmr[:], in0=mv[:, 0:1], scalar1=rstd[:], scalar2=-1.0,
                op0=mybir.AluOpType.mult, op1=mybir.AluOpType.mult)
            h_blk = scratch.tile([128, C], BF16, tag="h_blk")
            nc.scalar.activation(out=h_blk[:], in_=src_slice, func=AF.Identity,
                                 bias=nmr[:], scale=rstd[:])
            if g_rep is not None:
                nc.vector.tensor_mul(h_blk[:], h_blk[:], g_rep[:])
            if b_rep is not None:
                nc.vector.tensor_add(h_blk[:], h_blk[:], b_rep[:])
            return h_blk

        def transpose_into(ps_pool, dst, dst_ti, src_blk):
            for jc in range(NC_):
                pst = ps_pool.tile([128, 128], BF16, tag="ps1b")
                nc.tensor.transpose(pst[:], src_blk[:, jc * 128:(jc + 1) * 128],
                                    ident[:])
                nc.any.tensor_copy(
                    out=dst[:, jc, dst_ti * 128:(dst_ti + 1) * 128], in_=pst[:])

        # Global PSUM pools for stages 1-7: two 2-bank "main" slots (the
        # [128, <=1024] f32 accumulators: V, qkT(pairs 0-3), scores, proj,
        # fc1) + four 1-bank slots (transposes, PV halves, interleaved qkT
        # 512-chains, warmup) = exactly 8 banks, no stage barriers.
        ps_ctx = ExitStack()
        ps_main = ps_ctx.enter_context(
            tc.tile_pool(name="ps_main", bufs=3, space="PSUM"))
        ps_sm = ps_ctx.enter_context(
            tc.tile_pool(name="ps_sm", bufs=2, space="PSUM"))

        # ---- stage 1: warmup matmuls ---------------------------------------
        warm = ps_sm.tile([128, 128], F32, tag="ps1b", name="warm")
        for _ in range(96):
            nc.tensor.matmul(warm[:], ident[:], ident[:], start=True, stop=True)

        wqks_ctx = ExitStack()
        wqks = wqks_ctx.enter_context(tc.tile_pool(name="wqks", bufs=3))

        def qkT_chunk_full(sub, c):
            """Whole [128, T] chunk via a 2-bank psum (dense prologue form)."""
            m = sub * NC_ + c
            wq = wqks.tile([128, NC_, 128], BF16, tag="wq")
            nc.sync.dma_start(out=wq[:], in_=wqk_d[m])
            ps = ps_main.tile([128, T], F32, tag="psmain", name=f"psqk{m}")
            for k in range(NC_):
                for off, n in ((0, 512), (512, 512)):
                    nc.tensor.matmul(ps[:, off:off + n], wq[:, k, :],
                                     bufT[:, k, off:off + n],
                                     start=(k == 0), stop=(k == NC_ - 1))
            _evict_qk(ps, sub, c, m)

        def _evict_qk(ps, sub, c, m):
            if bqk_sb is not None:
                nc.scalar.activation(out=qk_full[:, sub, c, :], in_=ps[:],
                                     func=AF.Identity, bias=bqk_sb[:, m:m + 1])
            else:
                for off in (0, 512):
                    nc.vector.tensor_copy(
                        out=qk_full[:, sub, c, off:off + 512],
                        in_=ps[:, off:off + 512])

        def qkT_chunk_half(sub, c, half):
            """One [128, 512] half-chain via a 1-bank psum (attention filler)."""
            m = sub * NC_ + c
            if half == 0:
                wq = wqks.tile([128, NC_, 128], BF16, tag="wq",
                               name=f"wq_{m}")
                nc.sync.dma_start(out=wq[:], in_=wqk_d[m])
                qkT_chunk_half.cur[m] = wq
            wq = qkT_chunk_half.cur[m]
            off = half * 512
            ps = ps_sm.tile([128, 512], F32, tag="ps1b", name=f"psqk{m}_{half}")
            for k in range(NC_):
                nc.tensor.matmul(ps[:, 0:512], wq[:, k, :],
                                 bufT[:, k, off:off + 512],
                                 start=(k == 0), stop=(k == NC_ - 1))
            if bqk_sb is not None:
                nc.scalar.activation(out=qk_full[:, sub, c, off:off + 512],
                                     in_=ps[:], func=AF.Identity,
                                     bias=bqk_sb[:, m:m + 1])
            else:
                nc.vector.tensor_copy(out=qk_full[:, sub, c, off:off + 512],
                                      in_=ps[:])
        qkT_chunk_half.cur = {}

        # wp chunks: k=0..3 feed the proj partial pass that fills the tensor
        # engine during the ACT-bound pairs 4-7; k=4..7 DMA'd during
        # attention (own pool opened before `mid` so the DMA has no deps on
        # the attention pools' teardown).
        wplo_ctx = ExitStack()
        wplo = wplo_ctx.enter_context(tc.tile_pool(name="wplo", bufs=1))
        wp_lo = wplo.tile([128, 4, C], BF16, tag="wplo")
        for k in range(4):
            nc.sync.dma_start(out=wp_lo[:, k, :],
                              in_=wp_d[k * 128:(k + 1) * 128, :])
        wps_ctx = ExitStack()
        wps = wps_ctx.enter_context(tc.tile_pool(name="wps", bufs=1))
        wp_hi = wps.tile([128, 4, C], BF16, tag="wphi")
        for k in range(4):
            nc.sync.dma_start(out=wp_hi[:, k, :],
                              in_=wp_d[(k + 4) * 128:(k + 5) * 128, :])

        def proj_pass1(i):
            """r1[i] partial: += y[:, k0..3] @ Wp rows, via 1-bank psums."""
            for off in (0, 512):
                ps = ps_sm.tile([128, 512], F32, tag="ps1b", name=f"pj1_{i}_{off}")
                for k in range(4):
                    lhsT = yT[:, k, i * 128:(i + 1) * 128]
                    nc.tensor.matmul(ps[:], lhsT, wp_lo[:, k, off:off + 512],
                                     start=(k == 0), stop=(k == 3))
                nc.vector.tensor_add(x_sb[:, i, off:off + 512], ps[:],
                                     x_sb[:, i, off:off + 512])

        with tc.tile_pool(name="mid", bufs=1) as mid:
            vaug = mid.tile([128, NT, H, D + 1], BF16, tag="vaug")
            nc.vector.memset(vaug[:, :, :, D:D + 1], 1.0)

            # ---- stages 1b+2 fused: per token chunk: load x, LN1,
            # transpose -> h1T, then V(ti) right away so the tensor engine
            # stays dense through the LN latency chain.
            with tc.tile_pool(name="wvp", bufs=1) as wvp:
                wv_sb = wvp.tile([128, NC_, C], BF16, tag="wv")
                nc.sync.dma_start(out=x_sb[:, 0, :], in_=x_d[0:128, :])
                for k in range(NC_):
                    nc.sync.dma_start(out=wv_sb[:, k, :],
                                      in_=wv_d[k * 128:(k + 1) * 128, :])
                for ti in range(1, NT):
                    nc.sync.dma_start(out=x_sb[:, ti, :],
                                      in_=x_d[ti * 128:(ti + 1) * 128, :])
                for ti in range(NT):
                    h_blk = layernorm_chunk(x_sb[:, ti, :], ln1_g_rep,
                                            ln1_b_rep)
                    transpose_into(ps_sm, bufT, ti, h_blk)
                    ps = ps_main.tile([128, C], F32, tag="psmain", name=f"psv{ti}")
                    for k in range(NC_):
                        lhsT = bufT[:, k, ti * 128:(ti + 1) * 128]
                        for off, n in ((0, 512), (512, 512)):
                            nc.tensor.matmul(ps[:, off:off + n], lhsT,
                                             wv_sb[:, k, off:off + n],
                                             start=(k == 0), stop=(k == NC_ - 1))
                    if bv_rep is not None:
                        vs = scratch.tile([128, C], F32, tag="v_scr")
                        nc.vector.tensor_add(vs[:], ps[:], bv_rep[:])
                        vsrc = vs
                    else:
                        vsrc = ps
                    nc.vector.tensor_copy(
                        out=vaug[:, ti, :, 0:D],
                        in_=vsrc[:].rearrange("p (h d) -> p h d", d=D))

            # ---- stage 2b: qkT chunks for pairs 0-3 (dense) ----------------
            for c in range(4):
                for sub in range(2):
                    qkT_chunk_full(sub, c)

            # ---- stages 3+4: software-pipelined attention ------------------
            # Per pair c: scores for BOTH heads issued adjacently on 64-row
            # PE tiles (concurrent streams), qkT chunk c+4 half-chains as
            # PE filler, PV of pair c-1 lagged so the PE works while ACT
            # exponentiates pair c.
            _grps = ((0,), (1,), (2,), (3,), (4, 5), (6, 7))
            _dbs = [(0, 4), (4, 4), (8, 4), (12, 2), (14, 2)]

            def _dbatch(h):
                for b, (s, n) in enumerate(_dbs):
                    if s <= h < s + n:
                        return b, s
                raise AssertionError

            with tc.tile_pool(name="epool", bufs=4) as e_pool, \
                 tc.tile_pool(name="scrp", bufs=2) as scrp, \
                 tc.tile_pool(name="rbp", bufs=2) as rbp:
                den4s = [mid.tile([n, T], BF16, tag=f"den4_{b}", name=f"den4_{b}")
                         for b, (s, n) in enumerate(_dbs)]
                recip_dram = dram.tile([16, T], F32)
                egrps = {}   # h -> {j: (e_tile, col offset)}

                def scores(c):
                    """Scores+exp+mask for heads 2c (rows 0:64) and 2c+1
                    (rows 64:128), groups interleaved; yields between groups
                    so callers can emit PE filler."""
                    for h in (2 * c, 2 * c + 1):
                        egrps[h] = {}
                    for gi, grp in enumerate(_grps):
                        w_g = sum((8 - j) * 128 for j in grp)
                        for h in (2 * c, 2 * c + 1):
                            koff = (h % 2) * 64
                            ps = ps_main.tile([128, w_g], F32, tag="psmain",
                                              name=f"sp_{h}_{gi}")
                            col = 0
                            offs = []
                            for j in grp:
                                rem = (8 - j) * 128
                                lhsT = qk_full[koff:koff + 64, 1, c,
                                               j * 128:(j + 1) * 128]
                                off = col
                                src_off = j * 128
                                while off < col + rem:
                                    n = min(col + rem - off, 512 - (off % 512))
                                    nc.tensor.matmul(
                                        ps[:, off:off + n], lhsT,
                                        qk_full[koff:koff + 64, 0, c,
                                                src_off:src_off + n],
                                        start=True, stop=True)
                                    off += n
                                    src_off += n
                                offs.append(col)
                                col += rem
                            e = e_pool.tile([128, w_g], BF16, tag=f"e{gi}",
                                            name=f"e_{h}_{gi}")
                            nc.scalar.activation(out=e[:], in_=ps[:],
                                                 func=AF.Exp, scale=0.125)
                            for j, off in zip(grp, offs):
                                nc.vector.tensor_mul(
                                    e[:, off:off + 128], e[:, off:off + 128],
                                    mask_sb[:])
                                egrps[h][j] = (e, off)
                        yield gi

                def pv(c):
                    """PV + eviction + den bookkeeping for heads of pair c."""
                    for h in (2 * c, 2 * c + 1):
                        koff = (h % 2) * 64
                        egrp = egrps[h]
                        ps0 = ps_sm.tile([65, 512], F32, tag="ps1b",
                                         name=f"yt0_{h}")
                        ps1 = ps_sm.tile([65, 512], F32, tag="ps1b",
                                         name=f"yt1_{h}")
                        for j in range(NT):
                            lhsT = vaug[:, j, h, :]
                            et, eo = egrp[j]
                            if j <= 3:
                                nA = (4 - j) * 128
                                nc.tensor.matmul(
                                    ps0[:, j * 128:512], lhsT,
                                    et[:, eo:eo + nA],
                                    start=(j == 0), stop=(j == 3))
                                nc.tensor.matmul(
                                    ps1[:, 0:512], lhsT,
                                    et[:, eo + nA:eo + nA + 512],
                                    start=(j == 0), stop=False)
                            else:
                                nB = (8 - j) * 128
                                nc.tensor.matmul(
                                    ps1[:, j * 128 - 512:512], lhsT,
                                    et[:, eo:eo + nB],
                                    start=False, stop=(j == NT - 1))

                        scr = scrp.tile([65, T], BF16, tag="scr", name=f"scr_{h}")
                        nc.vector.tensor_copy(out=scr[:, 0:512], in_=ps0[:])
                        nc.vector.tensor_copy(out=scr[:, 512:1024], in_=ps1[:])
                        _b, _s = _dbatch(h)
                        nc.sync.dma_start(
                            out=den4s[_b][h - _s:h - _s + 1, :],
                            in_=scr[64:65, :])
                        nc.sync.dma_start(
                            out=yT[koff:koff + 64, h // 2, :], in_=scr[0:64, :])

                        batch = h in (3, 7, 11, 13, 15)
                        if batch:
                            b0 = {3: 0, 7: 4, 11: 8, 13: 12, 15: 14}[h]
                            bn = h - b0 + 1
                            recip4 = mid.tile([4, T], F32, tag="recip4")
                            _b2, _ = _dbatch(b0)
                            nc.vector.reciprocal(
                                out=recip4[0:bn, :],
                                in_=den4s[_b2][0:bn, :])
                            nc.sync.dma_start(
                                out=recip_dram[b0:b0 + bn, :],
                                in_=recip4[0:bn, :])
                            for hh in range(b0, b0 + bn):
                                ko2 = (hh % 2) * 64
                                rb = rbp.tile([128, T], F32, tag="rb",
                                              name=f"rb_{hh}")
                                rsrc = bass.AP(tensor=recip_dram.tensor,
                                               offset=hh * T,
                                               ap=[[0, 64], [1, T]])
                                nc.sync.dma_start(out=rb[ko2:ko2 + 64, :],
                                                  in_=rsrc)
                                nc.vector.tensor_mul(
                                    yT[ko2:ko2 + 64, hh // 2, :],
                                    yT[ko2:ko2 + 64, hh // 2, :],
                                    rb[ko2:ko2 + 64, :])

                # pipeline: scores(0); then for c: scores(c+1) groups with
                # qkT fillers, pv(c); tail pv(7).
                # filler schedule per pair c (c=0..3 -> qkT chunk c+4):
                # after groups 0,1,2,3 emit one 512 half-chain.
                for c in range(NC_):
                    sgen = scores(c)
                    for gi in sgen:
                        if c < 4:
                            if gi < 2:
                                qkT_chunk_half(gi, c + 4, 0)
                            elif gi < 4:
                                qkT_chunk_half(gi - 2, c + 4, 1)
                        if gi == 4 and c > 0:
                            pv(c - 1)
                    if c >= 4:
                        # yT k=0..3 are den-normalized once pv(3) ran (pair 4)
                        proj_pass1(2 * (c - 4))
                        proj_pass1(2 * (c - 4) + 1)
                    if c == NC_ - 1:
                        pv(c)

        # ---- stages 5+6 fused: finish proj (k 4-7) -> r1, LN2 -> h2T ------
        # transposes lag one chunk so the PE can run proj(i+1) while the
        # DVE/ACT layernorm chain of chunk i completes.
        h_blks = {}
        for i in range(NT):
            ps = ps_main.tile([128, C], F32, tag="psmain", name=f"pspj{i}")
            for k in range(4, NC_):
                lhsT = yT[:, k, i * 128:(i + 1) * 128]
                for off, n in ((0, 512), (512, 512)):
                    nc.tensor.matmul(ps[:, off:off + n], lhsT,
                                     wp_hi[:, k - 4, off:off + n],
                                     start=(k == 4), stop=(k == NC_ - 1))
            nc.vector.tensor_add(x_sb[:, i, :], ps[:], x_sb[:, i, :])
            if bproj_rep is not None:
                nc.vector.tensor_add(x_sb[:, i, :], x_sb[:, i, :],
                                     bproj_rep[:])
            h_blks[i] = layernorm_chunk(x_sb[:, i, :], ln2_g_rep, ln2_b_rep)
            if i > 0:
                transpose_into(ps_sm, bufT, i - 1, h_blks.pop(i - 1))
        transpose_into(ps_sm, bufT, NT - 1, h_blks.pop(NT - 1))
        wps_ctx.close()
        wplo_ctx.close()
        wqks_ctx.close()

        # ---- stage 7: fc1 + gelu -> aT ------------------------------------
        with tc.tile_pool(name="atp", bufs=1) as atp:
            aT = atp.tile([128, 4 * NC_, T], BF16, tag="aT")
            with tc.tile_pool(name="wfcs", bufs=3) as wfcs:
                for m in range(4 * NC_):
                    wf = wfcs.tile([128, NC_, 128], BF16, tag="wf")
                    nc.sync.dma_start(out=wf[:], in_=wfc_d[m])
                    ps = ps_main.tile([128, T], F32, tag="psmain",
                                      name=f"psf1_{m}")
                    for k in range(NC_):
                        for off, n in ((0, 512), (512, 512)):
                            nc.tensor.matmul(ps[:, off:off + n], wf[:, k, :],
                                             bufT[:, k, off:off + n],
                                             start=(k == 0), stop=(k == NC_ - 1))
                    bias = bfc_sb[:, m:m + 1] if bfc_sb is not None else 0.0
                    nc.scalar.activation(out=aT[:, m, :], in_=ps[:],
                                         func=AF.Gelu_apprx_tanh, bias=bias)

            # ---- stage 8: fc2 + residual -> out (2 column passes) ----------
            wm_ctx = ExitStack()
            wmlps = wm_ctx.enter_context(tc.tile_pool(name="wmlps", bufs=3))
            _pre = {}
            for pk in ((0, 0), (0, 1)):
                t = wmlps.tile([128, 512], BF16, tag="wm")
                nc.sync.dma_start(out=t[:], in_=wmlp_d[pk[0], pk[1]])
                _pre[pk] = t
            ps_ctx.close()
            with tc.tile_pool(name="ps_fc2", bufs=8, space="PSUM") as ps_fc2:
                for half in range(2):
                    hoff = half * 512
                    psums = {}
                    for i in range(NT):
                        psums[i] = ps_fc2.tile([128, 512], F32, tag="psf2",
                                               name=f"psf2_{half}_{i}")

                    def _evict(i):
                        o = o_pool.tile([128, 512], F32, tag="o")
                        nc.vector.tensor_add(o[:], psums[i][:],
                                             x_sb[:, i, hoff:hoff + 512])
                        if bmlp_rep is not None:
                            nc.vector.tensor_add(o[:], o[:],
                                                 bmlp_rep[:, hoff:hoff + 512])
                        nc.sync.dma_start(
                            out=out_d[i * 128:(i + 1) * 128, hoff:hoff + 512],
                            in_=o[:])

                    for k in range(4 * NC_):
                        last = k == 4 * NC_ - 1
                        wm = _pre.get((half, k))
                        if wm is None:
                            wm = wmlps.tile([128, 512], BF16, tag="wm")
                            nc.sync.dma_start(out=wm[:], in_=wmlp_d[half, k])
                        for i in range(NT):
                            lhsT = aT[:, k, i * 128:(i + 1) * 128]
                            nc.tensor.matmul(psums[i][:], lhsT, wm[:],
                                             start=(k == 0), stop=last)
                            if last:
                                _evict(i)
                wm_ctx.close()


# ---------------------------------------------------------------------------
_CACHE = {}


def _prearrange_kxm(w, nm):
    """[C, nm*128] -> [nm, 128, C//128, 128] bf16 so chunk DMAs are contiguous.

    out[m, p, ko, mm] = w[ko*128 + p, m*128 + mm]
    """
    cin = w.shape[0]
    a = w.reshape(cin // 128, 128, nm, 128)        # [ko, p, m, mm]
    a = np.transpose(a, (2, 1, 0, 3))              # [m, p, ko, mm]
    return np.ascontiguousarray(a.astype(ml_dtypes.bfloat16))


def _prearrange_mlp(w):
    """[4C, C] -> [2, 4C//128, 128, 512] bf16 column halves (fc2 passes)."""
    a = w.reshape(4 * NC_, 128, 2, 512)            # [k, p, half, n]
    a = np.transpose(a, (2, 0, 1, 3))              # [half, k, p, n]
    return np.ascontiguousarray(a.astype(ml_dtypes.bfloat16))


def _build_in_maps(inputs):
    x = np.asarray(inputs["x"], dtype=np.float32)
    w_qkv = np.asarray(inputs["w_qkv"], dtype=np.float32)
    b_qkv = np.asarray(inputs["b_qkv"], dtype=np.float32)

    flags = {
        "b_qk": bool(np.any(b_qkv[:2 * C])),
        "b_v": bool(np.any(b_qkv[2 * C:])),
        "b_proj": bool(np.any(inputs["b_attn_proj"])),
        "b_fc": bool(np.any(inputs["b_fc"])),
        "b_mlp": bool(np.any(inputs["b_mlp_proj"])),
        "ln1_g": not bool(np.allclose(np.asarray(inputs["ln1_g"]), 1.0)),
        "ln1_b": bool(np.any(inputs["ln1_b"])),
        "ln2_g": not bool(np.allclose(np.asarray(inputs["ln2_g"]), 1.0)),
        "ln2_b": bool(np.any(inputs["ln2_b"])),
    }

    bf = ml_dtypes.bfloat16
    shared = {
        "w_qk": _prearrange_kxm(w_qkv[:, :2 * C], 2 * NC_),
        "w_fc": _prearrange_kxm(np.asarray(inputs["w_fc"], np.float32), 4 * NC_),
        "w_v": np.ascontiguousarray(w_qkv[:, 2 * C:]).astype(bf),
        "w_proj": np.asarray(inputs["w_attn_proj"], np.float32).astype(bf),
        "w_mlp": _prearrange_mlp(np.asarray(inputs["w_mlp_proj"], np.float32)),
        "mask_ut": np.triu(np.ones((128, 128))).astype(bf),
    }
    if flags["b_qk"]:
        shared["b_qk"] = np.ascontiguousarray(b_qkv[:2 * C].reshape(2 * NC_, 128).T)
    if flags["b_v"]:
        shared["b_v"] = np.ascontiguousarray(b_qkv[2 * C:])
    if flags["b_proj"]:
        shared["b_proj"] = np.asarray(inputs["b_attn_proj"], np.float32)
    if flags["b_fc"]:
        shared["b_fc"] = np.ascontiguousarray(
            np.asarray(inputs["b_fc"], np.float32).reshape(4 * NC_, 128).T)
    if flags["b_mlp"]:
        shared["b_mlp"] = np.asarray(inputs["b_mlp_proj"], np.float32)
    for nm in ("ln1_g", "ln1_b", "ln2_g", "ln2_b"):
        if flags[nm]:
            shared[nm] = np.asarray(inputs[nm], np.float32)

    in_maps = [dict(shared, x=np.ascontiguousarray(x[c])) for c in range(x.shape[0])]
    return flags, in_maps


def kernel_run(inputs, trace=False, trace_kwargs=None):
    """Build (cached), run on 8 cores, return (full_output, BassKernelResults)."""
    from concourse.bass_utils import run_bass_kernel_spmd

    flags, in_maps = _build_in_maps(inputs)
    key = tuple(sorted(flags.items()))
    if key not in _CACHE:
        _CACHE[key] = build_nc(flags)
    nc = _CACHE[key]
    res = run_bass_kernel_spmd(nc, in_maps, core_ids=list(range(8)),
                               trace=trace, trace_kwargs=trace_kwargs or {})
    out = np.stack([res.results[c]["out"] for c in range(8)]).astype(np.float32)
    return out, res


def kernel(**inputs) -> np.ndarray:
    out, _ = kernel_run(inputs, trace=False)
    return out


# revision 29
# speedup vs baseline: 1.0498x; 1.0498x over previous
"""Trainium2 Bass kernel for a dense transformer block (B=8, T=1024, C=1024, H=16).

Data-parallel over batch across the 8 NeuronCores (one batch element per core,
weights replicated, no collectives).

Per-core dataflow (activations feature-major ("transposed") for matmuls,
token-major ("natural") for layernorm / softmax-denominator work):

  x [T,C] f32 --LN1(stats on DVE, normalize on ACT)--> h bf16 --PE T--> h1T
  v    = h @ Wv          (lhsT=h1T chunks, rhs=Wv)    -> natural [T, C] bf16
  qkT  = (h @ Wqk)^T     (lhsT=Wqk, rhs=h1T)          -> [2C, T] bf16
         chunks for head-pairs 0-3 upfront; chunks 4-7 interleaved into the
         attention loop as tensor-engine filler under the ACT-bound exp work.
  S^T  = k q^T           (lhsT=kT, rhs=qT, K=D=64)    -> [tk, tq] psum f32
         both heads of a pair issued back-to-back on 64-row PE tiles
         (tile_position (0,0)/(64,0)) so they stream concurrently.
  E^T  = exp(S^T/8) bf16 (no max-sub; scores ~N(0,1)); causal diag masked
         post-exp with a 0/1 upper-tri mask.
  y    = P @ [v | 1]     (lhsT=E^T blk, rhs=v_aug)    -> [65, tq] psum,
         lagged one pair behind scores so PE stays busy while ACT runs exp;
         col 64 = softmax denominator; divide via DRAM-broadcast recip.
  r1   = x + y @ Wp      (lhsT=yT)                    -> natural f32
  h2   = LN2(r1) --transpose--> h2T bf16
  aT   = gelu_tanh(Wfc^T h2)   (lhsT=Wfc, rhs=h2T)    -> [4C, T] bf16
  out  = r1 + a @ Wmlp   (lhsT=aT, rhs=Wmlp halves)   -> natural [T, C] f32
         two column passes, 8x 1-bank PSUMs, half the Wmlp DMA traffic.

All matmuls bf16 (full PE rate) with fp32 PSUM accumulation; LN statistics and
residual stream stay fp32.
"""
import sys

sys.path.insert(0, "/opt/trn_rl_repo")

import numpy as np
import ml_dtypes

import concourse.bass as bass
import concourse.tile as tile
from concourse import mybir
from concourse.masks import make_identity
from concourse.vector_clock import ScopedClock

F32 = mybir.dt.float32
BF16 = mybir.dt.bfloat16
AF = mybir.ActivationFunctionType

T, C, H, D = 1024, 1024, 16, 64
NT = T // 128   # 8 token chunks
NC_ = C // 128  # 8 feature chunks
EPS = 1e-5

# ---------------------------------------------------------------------------
# Walrus in this container rejects >1 sem-wait per CTRL instruction; split the
# Tile tail-drain's waits across nop carriers.
_MAX_WAITS = 1


def _patched_drain_and_barrier(self, tick_clock, wait_clock):
    nc = self.nc
    carrier = nc.sync.nop(nofuse=True)
    wait_clock.add_sem_waits(carrier.ins, ScopedClock({None: tick_clock.global_clock}))
    si = carrier.ins.sync_info
    waits = list(si.on_wait) if si and si.on_wait else []
    if len(waits) > _MAX_WAITS:
        si.on_wait = waits[:_MAX_WAITS]
        for k in range(_MAX_WAITS, len(waits), _MAX_WAITS):
            extra = nc.sync.nop(nofuse=True)
            esi = extra.ins.sync_info
            if esi is None:
                extra.ins.sync_info = mybir.SyncInfo(
                    on_wait=waits[k:k + _MAX_WAITS], on_update=[]
                )
            else:
                esi.on_wait = waits[k:k + _MAX_WAITS]
    nc.sync.drain()
    nc.all_engine_barrier()
    popped = nc._tile_sem_poison_stack.pop()
    assert popped is self._sem_poison
    nc.clear_and_free_semaphores(list(self.sems.allocated().values()))
    nc.all_engine_barrier()


tile.TileContext._drain_and_barrier = _patched_drain_and_barrier


def _split_sync_waits(nc, max_waits=1):
    """Walrus here rejects >1 sem-wait per instruction; hoist extras onto
    preceding same-engine nops."""
    ctr = 0
    for f in nc.m.functions:
        for b in f.blocks:
            out = []
            for ins in b.instructions:
                si = ins.sync_info
                ws = list(si.on_wait) if si and si.on_wait else []
                if len(ws) > max_waits:
                    extra, keep = ws[:-max_waits], ws[-max_waits:]
                    for i in range(0, len(extra), max_waits):
                        nop = mybir.InstNoOp(
                            name=f"wsplit-{ctr}", engine=ins.engine,
                            sync_info=mybir.SyncInfo(
                                on_wait=extra[i:i + max_waits], on_update=[]))
                        ctr += 1
                        out.append(nop)
                    si.on_wait = keep
                out.append(ins)
            b.instructions = out


def build_nc(flags):
    nc = bass.Bass()

    x_d = nc.dram_tensor("x", [T, C], F32, kind="ExternalInput")
    # host-prearranged: [m_chunk, p, ko, 128] so per-chunk DMAs are contiguous
    wqk_d = nc.dram_tensor("w_qk", [2 * NC_, 128, NC_, 128], BF16,
                           kind="ExternalInput")
    wfc_d = nc.dram_tensor("w_fc", [4 * NC_, 128, NC_, 128], BF16,
                           kind="ExternalInput")
    wv_d = nc.dram_tensor("w_v", [C, C], BF16, kind="ExternalInput")
    wp_d = nc.dram_tensor("w_proj", [C, C], BF16, kind="ExternalInput")
    # host-prearranged: [half, k, p, 512] column halves for the 2-pass fc2
    wmlp_d = nc.dram_tensor("w_mlp", [2, 4 * NC_, 128, 512], BF16,
                            kind="ExternalInput")
    mask_d = nc.dram_tensor("mask_ut", [128, 128], BF16, kind="ExternalInput")
    opt = {}
    if flags["b_qk"]:
        opt["b_qk"] = nc.dram_tensor("b_qk", [128, 2 * NC_], F32, kind="ExternalInput")
    if flags["b_v"]:
        opt["b_v"] = nc.dram_tensor("b_v", [C], F32, kind="ExternalInput")
    if flags["b_proj"]:
        opt["b_proj"] = nc.dram_tensor("b_proj", [C], F32, kind="ExternalInput")
    if flags["b_fc"]:
        opt["b_fc"] = nc.dram_tensor("b_fc", [128, 4 * NC_], F32, kind="ExternalInput")
    if flags["b_mlp"]:
        opt["b_mlp"] = nc.dram_tensor("b_mlp", [C], F32, kind="ExternalInput")
    for nm in ("ln1_g", "ln1_b", "ln2_g", "ln2_b"):
        if flags[nm]:
            opt[nm] = nc.dram_tensor(nm, [C], F32, kind="ExternalInput")
    out_d = nc.dram_tensor("out", [T, C], F32, kind="ExternalOutput")

    with tile.TileContext(nc) as tc:
        _build_body(nc, tc, flags, x_d, wqk_d, wv_d, wp_d, wfc_d, wmlp_d,
                    mask_d, opt, out_d)
    _split_sync_waits(nc)
    return nc


def _build_body(nc, tc, flags, x_d, wqk_d, wv_d, wp_d, wfc_d, wmlp_d,
                mask_d, opt, out_d):
    from contextlib import ExitStack

    ctx = ExitStack()
    with ctx:
        const = ctx.enter_context(tc.tile_pool(name="const", bufs=1))
        big = ctx.enter_context(tc.tile_pool(name="big", bufs=1))
        scratch = ctx.enter_context(tc.tile_pool(name="scratch", bufs=2))
        small = ctx.enter_context(tc.tile_pool(name="small", bufs=8))
        o_pool = ctx.enter_context(tc.tile_pool(name="opool", bufs=2))
        dram = ctx.enter_context(tc.tile_pool(name="dram", bufs=1, space="DRAM"))

        # ---- constants -----------------------------------------------------
        ident = const.tile([128, 128], BF16, tag="ident")
        make_identity(nc, ident)
        mask_sb = const.tile([128, 128], BF16, tag="mask")
        nc.sync.dma_start(mask_sb[:], mask_d[:])
        eps_t = const.tile([128, 1], F32, tag="eps")
        nc.vector.memset(eps_t[:], EPS)

        def rep128(vec_dram):
            t = const.tile([128, C], F32, tag=f"rep_{vec_dram.tensor.name}")
            src = bass.AP(tensor=vec_dram.tensor, offset=0, ap=[[0, 128], [1, C]])
            nc.gpsimd.dma_start(out=t[:], in_=src)
            return t

        ln1_g_rep = rep128(opt["ln1_g"]) if flags["ln1_g"] else None
        ln1_b_rep = rep128(opt["ln1_b"]) if flags["ln1_b"] else None
        ln2_g_rep = rep128(opt["ln2_g"]) if flags["ln2_g"] else None
        ln2_b_rep = rep128(opt["ln2_b"]) if flags["ln2_b"] else None
        bv_rep = rep128(opt["b_v"]) if flags["b_v"] else None
        bproj_rep = rep128(opt["b_proj"]) if flags["b_proj"] else None
        bmlp_rep = rep128(opt["b_mlp"]) if flags["b_mlp"] else None
        bqk_sb = None
        if flags["b_qk"]:
            bqk_sb = const.tile([128, 2 * NC_], F32, tag="bqk")
            nc.sync.dma_start(bqk_sb[:], opt["b_qk"][:])
        bfc_sb = None
        if flags["b_fc"]:
            bfc_sb = const.tile([128, 4 * NC_], F32, tag="bfc")
            nc.sync.dma_start(bfc_sb[:], opt["b_fc"][:])

        # ---- persistent tiles ---------------------------------------------
        x_sb = big.tile([128, NT, C], F32, tag="x")        # x, then r1 in place
        bufT = big.tile([128, NC_, T], BF16, tag="bufT")   # h1T -> h2T
        yT = big.tile([128, NC_, T], BF16, tag="yT")       # attention out^T
        # all qkT chunks: [p, sub(q=0,k=1), pair, t]
        qk_full = big.tile([128, 2, NC_, T], BF16, tag="qkf")

        def layernorm_chunk(src_slice, g_rep, b_rep):
            stats = small.tile([128, 2, 6], F32, tag="bn_stats")
            xr = src_slice.rearrange("p (s f) -> p s f", f=512)
            for s in range(2):
                nc.vector.bn_stats(out=stats[:, s, :], in_=xr[:, s, :])
            mv = small.tile([128, 2], F32, tag="bn_mv")
            nc.vector.bn_aggr(out=mv[:], in_=stats[:])
            rstd = small.tile([128, 1], F32, tag="rstd")
            nc.scalar.activation(out=rstd[:], in_=mv[:, 1:2], func=AF.Sqrt,
                                 bias=eps_t[:], scale=1.0)
            nc.vector.reciprocal(out=rstd[:], in_=rstd[:])
            # nmr = -mu * rstd; normalize on ACT: h = x*rstd + nmr
            nmr = small.tile([128, 1], F32, tag="nmr")
            nc.vector.tensor_scalar(
                out=nmr[:], in0=mv[:, 0:1], scalar1=rstd[:], scalar2=-1.0,
                op0=mybir.AluOpType.mult, op1=mybir.AluOpType.mult)
            h_blk = scratch.tile([128, C], BF16, tag="h_blk")
            nc.scalar.activation(out=h_blk[:], in_=src_slice, func=AF.Identity,
                                 bias=nmr[:], scale=rstd[:])
            if g_rep is not None:
                nc.vector.tensor_mul(h_blk[:], h_blk[:], g_rep[:])
            if b_rep is not None:
                nc.vector.tensor_add(h_blk[:], h_blk[:], b_rep[:])
            return h_blk

        def transpose_into(ps_pool, dst, dst_ti, src_blk):
            for jc in range(NC_):
                pst = ps_pool.tile([128, 128], BF16, tag="ps1b")
                nc.tensor.transpose(pst[:], src_blk[:, jc * 128:(jc + 1) * 128],
                                    ident[:])
                nc.any.tensor_copy(
                    out=dst[:, jc, dst_ti * 128:(dst_ti + 1) * 128], in_=pst[:])

        # Global PSUM pools for stages 1-7: two 2-bank "main" slots (the
        # [128, <=1024] f32 accumulators: V, qkT(pairs 0-3), scores, proj,
        # fc1) + four 1-bank slots (transposes, PV halves, interleaved qkT
        # 512-chains, warmup) = exactly 8 banks, no stage barriers.
        ps_ctx = ExitStack()
        ps_main = ps_ctx.enter_context(
            tc.tile_pool(name="ps_main", bufs=3, space="PSUM"))
        ps_sm = ps_ctx.enter_context(
            tc.tile_pool(name="ps_sm", bufs=2, space="PSUM"))

        # ---- stage 1: warmup matmuls ---------------------------------------
        warm = ps_sm.tile([128, 128], F32, tag="ps1b", name="warm")
        for _ in range(96):
            nc.tensor.matmul(warm[:], ident[:], ident[:], start=True, stop=True)

        wqks_ctx = ExitStack()
        wqks = wqks_ctx.enter_context(tc.tile_pool(name="wqks", bufs=3))

        def qkT_chunk_full(sub, c):
            """Whole [128, T] chunk via a 2-bank psum (dense prologue form)."""
            m = sub * NC_ + c
            wq = wqks.tile([128, NC_, 128], BF16, tag="wq")
            nc.sync.dma_start(out=wq[:], in_=wqk_d[m])
            ps = ps_main.tile([128, T], F32, tag="psmain", name=f"psqk{m}")
            for k in range(NC_):
                for off, n in ((0, 512), (512, 512)):
                    nc.tensor.matmul(ps[:, off:off + n], wq[:, k, :],
                                     bufT[:, k, off:off + n],
                                     start=(k == 0), stop=(k == NC_ - 1))
            _evict_qk(ps, sub, c, m)

        def _evict_qk(ps, sub, c, m):
            if bqk_sb is not None:
                nc.scalar.activation(out=qk_full[:, sub, c, :], in_=ps[:],
                                     func=AF.Identity, bias=bqk_sb[:, m:m + 1])
            else:
                for off in (0, 512):
                    nc.vector.tensor_copy(
                        out=qk_full[:, sub, c, off:off + 512],
                        in_=ps[:, off:off + 512])

        def qkT_chunk_half(sub, c, half):
            """One [128, 512] half-chain via a 1-bank psum (attention filler)."""
            m = sub * NC_ + c
            if half == 0:
                wq = wqks.tile([128, NC_, 128], BF16, tag="wq",
                               name=f"wq_{m}")
                nc.sync.dma_start(out=wq[:], in_=wqk_d[m])
                qkT_chunk_half.cur[m] = wq
            wq = qkT_chunk_half.cur[m]
            off = half * 512
            ps = ps_sm.tile([128, 512], F32, tag="ps1b", name=f"psqk{m}_{half}")
            for k in range(NC_):
                nc.tensor.matmul(ps[:, 0:512], wq[:, k, :],
                                 bufT[:, k, off:off + 512],
                                 start=(k == 0), stop=(k == NC_ - 1))
            if bqk_sb is not None:
                nc.scalar.activation(out=qk_full[:, sub, c, off:off + 512],
                                     in_=ps[:], func=AF.Identity,
                                     bias=bqk_sb[:, m:m + 1])
            else:
                nc.vector.tensor_copy(out=qk_full[:, sub, c, off:off + 512],
                                      in_=ps[:])
        qkT_chunk_half.cur = {}

        # wp chunks: k=0..3 feed the proj partial pass that fills the tensor
        # engine during the ACT-bound pairs 4-7; k=4..7 DMA'd during
        # attention (own pool opened before `mid` so the DMA has no deps on
        # the attention pools' teardown).
        wplo_ctx = ExitStack()
        wplo = wplo_ctx.enter_context(tc.tile_pool(name="wplo", bufs=1))
        wp_lo = wplo.tile([128, 4, C], BF16, tag="wplo")
        wps_ctx = ExitStack()
        wps = wps_ctx.enter_context(tc.tile_pool(name="wps", bufs=1))
        wp_hi = wps.tile([128, 4, C], BF16, tag="wphi")

        def proj_pass1(i):
            """r1[i] partial: += y[:, k0..3] @ Wp rows, via 1-bank psums."""
            for off in (0, 512):
                ps = ps_sm.tile([128, 512], F32, tag="ps1b", name=f"pj1_{i}_{off}")
                for k in range(4):
                    lhsT = yT[:, k, i * 128:(i + 1) * 128]
                    nc.tensor.matmul(ps[:], lhsT, wp_lo[:, k, off:off + 512],
                                     start=(k == 0), stop=(k == 3))
                nc.vector.tensor_add(x_sb[:, i, off:off + 512], ps[:],
                                     x_sb[:, i, off:off + 512])

        with tc.tile_pool(name="mid", bufs=1) as mid:
            vaug = mid.tile([128, NT, H, D + 1], BF16, tag="vaug")
            nc.vector.memset(vaug[:, :, :, D:D + 1], 1.0)

            # ---- stages 1b+2 fused: per token chunk: load x, LN1,
            # transpose -> h1T, then V(ti) right away so the tensor engine
            # stays dense through the LN latency chain.
            with tc.tile_pool(name="wvp", bufs=1) as wvp:
                wv_sb = wvp.tile([128, NC_, C], BF16, tag="wv")
                nc.sync.dma_start(out=x_sb[:, 0, :], in_=x_d[0:128, :])
                for k in range(NC_):
                    nc.sync.dma_start(out=wv_sb[:, k, :],
                                      in_=wv_d[k * 128:(k + 1) * 128, :])
                for ti in range(1, NT):
                    nc.sync.dma_start(out=x_sb[:, ti, :],
                                      in_=x_d[ti * 128:(ti + 1) * 128, :])
                for k in range(4):
                    nc.sync.dma_start(out=wp_lo[:, k, :],
                                      in_=wp_d[k * 128:(k + 1) * 128, :])
                    nc.sync.dma_start(out=wp_hi[:, k, :],
                                      in_=wp_d[(k + 4) * 128:(k + 5) * 128, :])
                for ti in range(NT):
                    h_blk = layernorm_chunk(x_sb[:, ti, :], ln1_g_rep,
                                            ln1_b_rep)
                    transpose_into(ps_sm, bufT, ti, h_blk)
                    ps = ps_main.tile([128, C], F32, tag="psmain", name=f"psv{ti}")
                    for k in range(NC_):
                        lhsT = bufT[:, k, ti * 128:(ti + 1) * 128]
                        for off, n in ((0, 512), (512, 512)):
                            nc.tensor.matmul(ps[:, off:off + n], lhsT,
                                             wv_sb[:, k, off:off + n],
                                             start=(k == 0), stop=(k == NC_ - 1))
                    if bv_rep is not None:
                        vs = scratch.tile([128, C], F32, tag="v_scr")
                        nc.vector.tensor_add(vs[:], ps[:], bv_rep[:])
                        vsrc = vs
                    else:
                        vsrc = ps
                    nc.vector.tensor_copy(
                        out=vaug[:, ti, :, 0:D],
                        in_=vsrc[:].rearrange("p (h d) -> p h d", d=D))

            # ---- stage 2b: qkT chunks for pairs 0-3 (dense) ----------------
            for c in range(4):
                for sub in range(2):
                    qkT_chunk_full(sub, c)

            # ---- stages 3+4: software-pipelined attention ------------------
            # Per pair c: scores for BOTH heads issued adjacently on 64-row
            # PE tiles (concurrent streams), qkT chunk c+4 half-chains as
            # PE filler, PV of pair c-1 lagged so the PE works while ACT
            # exponentiates pair c.
            _grps = ((0,), (1,), (2,), (3,), (4, 5), (6, 7))
            _dbs = [(0, 4), (4, 4), (8, 4), (12, 2), (14, 2)]

            def _dbatch(h):
                for b, (s, n) in enumerate(_dbs):
                    if s <= h < s + n:
                        return b, s
                raise AssertionError

            with tc.tile_pool(name="epool", bufs=4) as e_pool, \
                 tc.tile_pool(name="scrp", bufs=2) as scrp, \
                 tc.tile_pool(name="rbp", bufs=2) as rbp:
                den4s = [mid.tile([n, T], BF16, tag=f"den4_{b}", name=f"den4_{b}")
                         for b, (s, n) in enumerate(_dbs)]
                recip_dram = dram.tile([16, T], BF16)
                egrps = {}   # h -> {j: (e_tile, col offset)}

                def _den_batch(b0, h_last):
                    bn = h_last - b0 + 1
                    _b2, _ = _dbatch(b0)
                    with nc.allow_low_precision(
                            reason="softmax denom recip in bf16: denom is "
                                   "O(64-512), 0.4% rel err fits the budget"):
                        nc.vector.reciprocal(
                            out=den4s[_b2][0:bn, :], in_=den4s[_b2][0:bn, :])
                    nc.sync.dma_start(
                        out=recip_dram[b0:b0 + bn, :],
                        in_=den4s[_b2][0:bn, :])
                    for hh in range(b0, b0 + bn):
                        ko2 = (hh % 2) * 64
                        rb = rbp.tile([128, T], BF16, tag="rb",
                                      name=f"rb_{hh}")
                        rsrc = bass.AP(tensor=recip_dram.tensor,
                                       offset=hh * T,
                                       ap=[[0, 64], [1, T]])
                        nc.sync.dma_start(out=rb[ko2:ko2 + 64, :],
                                          in_=rsrc)
                        nc.vector.tensor_mul(
                            yT[ko2:ko2 + 64, hh // 2, :],
                            yT[ko2:ko2 + 64, hh // 2, :],
                            rb[ko2:ko2 + 64, :])

                def scores(c):
                    """Scores+exp+mask for heads 2c (rows 0:64) and 2c+1
                    (rows 64:128), groups interleaved; yields between groups
                    so callers can emit PE filler."""
                    for h in (2 * c, 2 * c + 1):
                        egrps[h] = {}
                    for gi, grp in enumerate(_grps):
                        w_g = sum((8 - j) * 128 for j in grp)
                        for h in (2 * c, 2 * c + 1):
                            koff = (h % 2) * 64
                            ps = ps_main.tile([128, w_g], F32, tag="psmain",
                                              name=f"sp_{h}_{gi}")
                            col = 0
                            offs = []
                            for j in grp:
                                rem = (8 - j) * 128
                                lhsT = qk_full[koff:koff + 64, 1, c,
                                               j * 128:(j + 1) * 128]
                                off = col
                                src_off = j * 128
                                while off < col + rem:
                                    n = min(col + rem - off, 512 - (off % 512))
                                    nc.tensor.matmul(
                                        ps[:, off:off + n], lhsT,
                                        qk_full[koff:koff + 64, 0, c,
                                                src_off:src_off + n],
                                        start=True, stop=True)
                                    off += n
                                    src_off += n
                                offs.append(col)
                                col += rem
                            e = e_pool.tile([128, w_g], BF16, tag=f"e{gi}",
                                            name=f"e_{h}_{gi}")
                            nc.scalar.activation(out=e[:], in_=ps[:],
                                                 func=AF.Exp, scale=0.125)
                            for j, off in zip(grp, offs):
                                nc.vector.tensor_mul(
                                    e[:, off:off + 128], e[:, off:off + 128],
                                    mask_sb[:])
                                egrps[h][j] = (e, off)
                        yield gi

                def pv(c):
                    """PV + eviction + den bookkeeping for heads of pair c."""
                    for h in (2 * c, 2 * c + 1):
                        koff = (h % 2) * 64
                        egrp = egrps[h]
                        ps0 = ps_sm.tile([65, 512], F32, tag="ps1b",
                                         name=f"yt0_{h}")
                        ps1 = ps_sm.tile([65, 512], F32, tag="ps1b",
                                         name=f"yt1_{h}")
                        for j in range(NT):
                            lhsT = vaug[:, j, h, :]
                            et, eo = egrp[j]
                            if j <= 3:
                                nA = (4 - j) * 128
                                nc.tensor.matmul(
                                    ps0[:, j * 128:512], lhsT,
                                    et[:, eo:eo + nA],
                                    start=(j == 0), stop=(j == 3))
                                nc.tensor.matmul(
                                    ps1[:, 0:512], lhsT,
                                    et[:, eo + nA:eo + nA + 512],
                                    start=(j == 0), stop=False)
                            else:
                                nB = (8 - j) * 128
                                nc.tensor.matmul(
                                    ps1[:, j * 128 - 512:512], lhsT,
                                    et[:, eo:eo + nB],
                                    start=False, stop=(j == NT - 1))

                        scr = scrp.tile([65, T], BF16, tag="scr", name=f"scr_{h}")
                        nc.vector.tensor_copy(out=scr[:, 0:512], in_=ps0[:])
                        nc.vector.tensor_copy(out=scr[:, 512:1024], in_=ps1[:])
                        _b, _s = _dbatch(h)
                        nc.sync.dma_start(
                            out=den4s[_b][h - _s:h - _s + 1, :],
                            in_=scr[64:65, :])
                        nc.sync.dma_start(
                            out=yT[koff:koff + 64, h // 2, :], in_=scr[0:64, :])

                        if h in (3, 7):
                            _den_batch({3: 0, 7: 4}[h], h)

                # pipeline: scores(0); then for c: scores(c+1) groups with
                # qkT fillers, pv(c); tail pv(7).
                # filler schedule per pair c (c=0..3 -> qkT chunk c+4):
                # after groups 0,1,2,3 emit one 512 half-chain.
                for c in range(NC_):
                    sgen = scores(c)
                    for gi in sgen:
                        if c < 4:
                            if gi < 2:
                                qkT_chunk_half(gi, c + 4, 0)
                            elif gi < 4:
                                qkT_chunk_half(gi - 2, c + 4, 1)
                        if gi == 4 and c > 0:
                            pv(c - 1)
                    if c >= 4:
                        # yT k=0..3 are den-normalized once pv(3) ran (pair 4)
                        proj_pass1(2 * (c - 4))
                        proj_pass1(2 * (c - 4) + 1)
                    if c == NC_ - 1:
                        pv(c)
                # heads 8-15: den normalization deferred out of the pair loop
                # so the DVE never head-of-line blocks on the DRAM-broadcast
                # DMAs mid-attention; the stage-5 k4-6 runway hides this tail.
                _den_batch(8, 11)
                _den_batch(12, 13)
                _den_batch(14, 15)

        # ---- stages 5+6 fused: finish proj (k 4-7) -> r1, LN2 -> h2T ------
        # Chains run 2 deep with the k=7 matmuls lagged, so the PE has a
        # k=4..6 runway while the deferred den tail (which gates yT k=7)
        # completes; LN2 transposes lag one more chunk behind the DVE/ACT
        # layernorm chain.
        h_blks = {}
        chains = {}

        def _finish(j):
            ps = chains.pop(j)
            lhsT = yT[:, NC_ - 1, j * 128:(j + 1) * 128]
            for off, n in ((0, 512), (512, 512)):
                nc.tensor.matmul(ps[:, off:off + n], lhsT,
                                 wp_hi[:, 3, off:off + n],
                                 start=False, stop=True)
            nc.vector.tensor_add(x_sb[:, j, :], ps[:], x_sb[:, j, :])
            if bproj_rep is not None:
                nc.vector.tensor_add(x_sb[:, j, :], x_sb[:, j, :],
                                     bproj_rep[:])
            h_blks[j] = layernorm_chunk(x_sb[:, j, :], ln2_g_rep, ln2_b_rep)
            if j > 0:
                transpose_into(ps_sm, bufT, j - 1, h_blks.pop(j - 1))

        for i in range(NT):
            ps = ps_main.tile([128, C], F32, tag="psmain", name=f"pspj{i}")
            chains[i] = ps
            for k in range(4, NC_ - 1):
                lhsT = yT[:, k, i * 128:(i + 1) * 128]
                for off, n in ((0, 512), (512, 512)):
                    nc.tensor.matmul(ps[:, off:off + n], lhsT,
                                     wp_hi[:, k - 4, off:off + n],
                                     start=(k == 4), stop=False)
            if i >= 1:
                _finish(i - 1)
        _finish(NT - 1)
        transpose_into(ps_sm, bufT, NT - 1, h_blks.pop(NT - 1))
        wps_ctx.close()
        wplo_ctx.close()
        wqks_ctx.close()

        # ---- stage 7: fc1 + gelu -> aT ------------------------------------
        with tc.tile_pool(name="atp", bufs=1) as atp:
            aT = atp.tile([128, 4 * NC_, T], BF16, tag="aT")
            with tc.tile_pool(name="wfcs", bufs=3) as wfcs:
                for m in range(4 * NC_):
                    wf = wfcs.tile([128, NC_, 128], BF16, tag="wf")
                    nc.sync.dma_start(out=wf[:], in_=wfc_d[m])
                    ps = ps_main.tile([128, T], F32, tag="psmain",
                                      name=f"psf1_{m}")
                    for k in range(NC_):
                        for off, n in ((0, 512), (512, 512)):
                            nc.tensor.matmul(ps[:, off:off + n], wf[:, k, :],
                                             bufT[:, k, off:off + n],
                                             start=(k == 0), stop=(k == NC_ - 1))
                    bias = bfc_sb[:, m:m + 1] if bfc_sb is not None else 0.0
                    nc.scalar.activation(out=aT[:, m, :], in_=ps[:],
                                         func=AF.Gelu_apprx_tanh, bias=bias)

            # ---- stage 8: fc2 + residual -> out (2 column passes) ----------
            wm_ctx = ExitStack()
            wmlps = wm_ctx.enter_context(tc.tile_pool(name="wmlps", bufs=3))
            _pre = {}
            for pk in ((0, 0), (0, 1)):
                t = wmlps.tile([128, 512], BF16, tag="wm")
                nc.sync.dma_start(out=t[:], in_=wmlp_d[pk[0], pk[1]])
                _pre[pk] = t
            ps_ctx.close()
            with tc.tile_pool(name="ps_fc2", bufs=8, space="PSUM") as ps_fc2:
                for half in range(2):
                    hoff = half * 512
                    psums = {}
                    for i in range(NT):
                        psums[i] = ps_fc2.tile([128, 512], F32, tag="psf2",
                                               name=f"psf2_{half}_{i}")

                    def _evict(i):
                        o = o_pool.tile([128, 512], F32, tag="o")
                        nc.vector.tensor_add(o[:], psums[i][:],
                                             x_sb[:, i, hoff:hoff + 512])
                        if bmlp_rep is not None:
                            nc.vector.tensor_add(o[:], o[:],
                                                 bmlp_rep[:, hoff:hoff + 512])
                        nc.sync.dma_start(
                            out=out_d[i * 128:(i + 1) * 128, hoff:hoff + 512],
                            in_=o[:])

                    for k in range(4 * NC_):
                        last = k == 4 * NC_ - 1
                        wm = _pre.get((half, k))
                        if wm is None:
                            wm = wmlps.tile([128, 512], BF16, tag="wm")
                            nc.sync.dma_start(out=wm[:], in_=wmlp_d[half, k])
                        for i in range(NT):
                            lhsT = aT[:, k, i * 128:(i + 1) * 128]
                            nc.tensor.matmul(psums[i][:], lhsT, wm[:],
                                             start=(k == 0), stop=last)
                            if last:
                                _evict(i)
                wm_ctx.close()


# ---------------------------------------------------------------------------
_CACHE = {}


def _prearrange_kxm(w, nm):
    """[C, nm*128] -> [nm, 128, C//128, 128] bf16 so chunk DMAs are contiguous.

    out[m, p, ko, mm] = w[ko*128 + p, m*128 + mm]
    """
    cin = w.shape[0]
    a = w.reshape(cin // 128, 128, nm, 128)        # [ko, p, m, mm]
    a = np.transpose(a, (2, 1, 0, 3))              # [m, p, ko, mm]
    return np.ascontiguousarray(a.astype(ml_dtypes.bfloat16))


def _prearrange_mlp(w):
    """[4C, C] -> [2, 4C//128, 128, 512] bf16 column halves (fc2 passes)."""
    a = w.reshape(4 * NC_, 128, 2, 512)            # [k, p, half, n]
    a = np.transpose(a, (2, 0, 1, 3))              # [half, k, p, n]
    return np.ascontiguousarray(a.astype(ml_dtypes.bfloat16))


def _build_in_maps(inputs):
    x = np.asarray(inputs["x"], dtype=np.float32)
    w_qkv = np.asarray(inputs["w_qkv"], dtype=np.float32)
    b_qkv = np.asarray(inputs["b_qkv"], dtype=np.float32)

    flags = {
        "b_qk": bool(np.any(b_qkv[:2 * C])),
        "b_v": bool(np.any(b_qkv[2 * C:])),
        "b_proj": bool(np.any(inputs["b_attn_proj"])),
        "b_fc": bool(np.any(inputs["b_fc"])),
        "b_mlp": bool(np.any(inputs["b_mlp_proj"])),
        "ln1_g": not bool(np.allclose(np.asarray(inputs["ln1_g"]), 1.0)),
        "ln1_b": bool(np.any(inputs["ln1_b"])),
        "ln2_g": not bool(np.allclose(np.asarray(inputs["ln2_g"]), 1.0)),
        "ln2_b": bool(np.any(inputs["ln2_b"])),
    }

    bf = ml_dtypes.bfloat16
    shared = {
        "w_qk": _prearrange_kxm(w_qkv[:, :2 * C], 2 * NC_),
        "w_fc": _prearrange_kxm(np.asarray(inputs["w_fc"], np.float32), 4 * NC_),
        "w_v": np.ascontiguousarray(w_qkv[:, 2 * C:]).astype(bf),
        "w_proj": np.asarray(inputs["w_attn_proj"], np.float32).astype(bf),
        "w_mlp": _prearrange_mlp(np.asarray(inputs["w_mlp_proj"], np.float32)),
        "mask_ut": np.triu(np.ones((128, 128))).astype(bf),
    }
    if flags["b_qk"]:
        shared["b_qk"] = np.ascontiguousarray(b_qkv[:2 * C].reshape(2 * NC_, 128).T)
    if flags["b_v"]:
        shared["b_v"] = np.ascontiguousarray(b_qkv[2 * C:])
    if flags["b_proj"]:
        shared["b_proj"] = np.asarray(inputs["b_attn_proj"], np.float32)
    if flags["b_fc"]:
        shared["b_fc"] = np.ascontiguousarray(
            np.asarray(inputs["b_fc"], np.float32).reshape(4 * NC_, 128).T)
    if flags["b_mlp"]:
        shared["b_mlp"] = np.asarray(inputs["b_mlp_proj"], np.float32)
    for nm in ("ln1_g", "ln1_b", "ln2_g", "ln2_b"):
        if flags[nm]:
            shared[nm] = np.asarray(inputs[nm], np.float32)

    in_maps = [dict(shared, x=np.ascontiguousarray(x[c])) for c in range(x.shape[0])]
    return flags, in_maps


def kernel_run(inputs, trace=False, trace_kwargs=None):
    """Build (cached), run on 8 cores, return (full_output, BassKernelResults)."""
    from concourse.bass_utils import run_bass_kernel_spmd

    flags, in_maps = _build_in_maps(inputs)
    key = tuple(sorted(flags.items()))
    if key not in _CACHE:
        _CACHE[key] = build_nc(flags)
    nc = _CACHE[key]
    res = run_bass_kernel_spmd(nc, in_maps, core_ids=list(range(8)),
                               trace=trace, trace_kwargs=trace_kwargs or {})
    out = np.stack([res.results[c]["out"] for c in range(8)]).astype(np.float32)
    return out, res


def kernel(**inputs) -> np.ndarray:
    out, _ = kernel_run(inputs, trace=False)
    return out
